# revision 36
# baseline (speedup 1.0000x reference)
"""2-layer GAT (PyG GATConv-style) on 8 Trainium2 NeuronCores.

Sharding (dst-tile blocks): nodes padded to 160 tiles of 128; core c owns
dst-tiles [20c, 20c+20). Edges (incl. self-loops) live on the core owning
their destination, sorted by dst-tile then src, padded to 128-edge chunks.
Dense GEMM1 is replicated in bf16; per-edge rows ([h|asrc] bf16, 768B
stride) are fetched with GPSIMD dma_gather from core-local HBM; gather
descriptor generation (~9ns/idx of Q7 ucode) is the critical resource and
paces both edge phases. (An optional prepare_only/trigger_dma pipeline is
behind GAT_PREP=1 but measured slower.) Segment softmax and
scatter-aggregate run per dst-tile as bf16 one-hot matmuls with s01/t01
streamed from HBM. Between layers the per-node [h2@W2|asrc2] rows are
exchanged with 4 chunked AllGathers that overlap the tail of layer-1
compute; layer-2 gather indices are host-side remapped into the chunked
layout.
"""

import numpy as np
import ml_dtypes

import concourse.bass as bass
import concourse.mybir as mybir
import concourse.tile as tile
from concourse import bacc
from concourse.bass_utils import run_bass_kernel_spmd

dt = mybir.dt
bf16 = ml_dtypes.bfloat16

N_CORES = 8
N_NODES = 20000
IN_CH = 128
HID = 32
HEADS = 8
HC = HEADS * HID  # 256
OUT_CH = 64
NEG_SLOPE = 0.2

P = 128
N_TILES_TOTAL = 157  # ceil(20000/128)
TILES_PER_CORE = 20  # 8*20 = 160 >= 157
N_PAD = 160 * P      # 20480
LOC_NODES = TILES_PER_CORE * P  # 2560

ROW1 = 384   # Hbuf row stride: [H(256) | asrc(8) | pad] bf16 (768B)
ROW1D = 264  # data portion of a Hbuf row
ROW2 = 128   # h2 row stride: [hW2(64) | asrc2(1) | pad] bf16 (256B)
ROW2D = 65   # data portion of a h2 row
TS = [5, 5, 5, 5]   # allgather chunk sizes (tiles)
CUM = [0]
for _t in TS:
    CUM.append(CUM[-1] + _t)
NCHUNK = len(TS)
import os as _os
PA1 = int(_os.environ.get("GAT_PA1", "4"))  # layer-1 gather pipeline depth
PA2 = int(_os.environ.get("GAT_PA2", "6"))  # layer-2 gather pipeline depth
import os
SINGLE_PACKET = os.environ.get("GAT_SP", "") != ""
PREP_PIPELINE = os.environ.get("GAT_PREP", "") != ""  # prepare_only pipelining
PREP2 = PREP_PIPELINE and os.environ.get("GAT_NOPREP2", "") == ""  # layer-2 too

AF = mybir.ActivationFunctionType
OP = mybir.AluOpType


def _remap_node(s):
    """Node id -> row in the chunked h2all layout."""
    c = s // LOC_NODES
    j = (s % LOC_NODES) // P
    p = s % P
    k = np.searchsorted(np.asarray(CUM), j, side="right") - 1
    cum = np.asarray(CUM)[k]
    ts = np.asarray(TS)[k]
    return (cum * N_CORES * P + c * ts * P + (j - cum) * P + p)


def _pack_idx(sp):
    n_pad = len(sp)
    idx16 = sp.astype(np.int16).reshape(n_pad // 16, 16).T
    return np.tile(idx16, (8, 1))


def _prep_edges(edge_index):
    src = np.asarray(edge_index[0], dtype=np.int64)
    dst = np.asarray(edge_index[1], dtype=np.int64)
    loops = np.arange(N_NODES, dtype=np.int64)
    src = np.concatenate([src, loops])
    dst = np.concatenate([dst, loops])

    order = np.lexsort((src, dst))
    src, dst = src[order], dst[order]
    tile_of = dst // P
    core_of = np.minimum(tile_of // TILES_PER_CORE, N_CORES - 1)

    per = [[None] * TILES_PER_CORE for _ in range(N_CORES)]
    for c in range(N_CORES):
        mc = core_of == c
        sc, tc_, dc = src[mc], tile_of[mc], dst[mc]
        for j in range(TILES_PER_CORE):
            gt = c * TILES_PER_CORE + j
            mt = tc_ == gt
            per[c][j] = (sc[mt], dc[mt] - gt * P)

    G = [max(max(1, -(-len(per[c][j][0]) // P)) for c in range(N_CORES))
         for j in range(TILES_PER_CORE)]

    meta = []
    for c in range(N_CORES):
        idx_cols, idx2_cols, s01s, t01s = [], [], [], []
        for j in range(TILES_PER_CORE):
            s, dl = per[c][j]
            n_pad = G[j] * P
            sp = np.zeros(n_pad, dtype=np.int64)
            sp[: len(s)] = s
            dlp = np.full(n_pad, 200, dtype=np.int64)
            dlp[: len(dl)] = dl
            idx_cols.append(_pack_idx(sp))
            idx2_cols.append(_pack_idx(_remap_node(sp)))
            for g in range(G[j]):
                ed = dlp[g * P:(g + 1) * P]
                s01 = (ed[:, None] == np.arange(P)[None, :])
                s01s.append(s01.astype(bf16))
                t01s.append(np.ascontiguousarray(s01.T).astype(bf16))
        meta.append({
            "idx": np.ascontiguousarray(np.concatenate(idx_cols, axis=1)),
            "idx2": np.ascontiguousarray(np.concatenate(idx2_cols, axis=1)),
            "s01": np.ascontiguousarray(np.stack(s01s, 1).reshape(P, -1)),
            "t01": np.ascontiguousarray(np.stack(t01s, 1).reshape(P, -1)),
        })
    return G, meta


def _build_program(G):
    NCH = sum(G)
    Gmax = max(G)
    nc = bacc.Bacc(None, target_bir_lowering=False, debug=False,
                   num_swdge_queues=2)

    xT = nc.dram_tensor("xT", [P, N_PAD], dt.bfloat16, kind="ExternalInput")
    xTloc = nc.dram_tensor("xTloc", [P, LOC_NODES], dt.bfloat16, kind="ExternalInput")
    W1a = nc.dram_tensor("W1a", [P, HC + 16], dt.bfloat16, kind="ExternalInput")
    W2a = nc.dram_tensor("W2a", [HC, OUT_CH + 2], dt.bfloat16, kind="ExternalInput")
    b1r = nc.dram_tensor("b1r", [P, HC], dt.float32, kind="ExternalInput")
    b2r = nc.dram_tensor("b2r", [P, OUT_CH], dt.float32, kind="ExternalInput")
    idxT = nc.dram_tensor("idx", [P, 8 * NCH], dt.int16, kind="ExternalInput")
    idx2T = nc.dram_tensor("idx2", [P, 8 * NCH], dt.int16, kind="ExternalInput")
    s01T = nc.dram_tensor("s01", [P, NCH * P], dt.bfloat16, kind="ExternalInput")
    t01T = nc.dram_tensor("t01", [P, NCH * P], dt.bfloat16, kind="ExternalInput")
    outT = nc.dram_tensor("out", [LOC_NODES, OUT_CH], dt.float32, kind="ExternalOutput")

    coff = [0]
    for j in range(1, TILES_PER_CORE + 1):
        coff.append(coff[-1] + G[j - 1])

    with tile.TileContext(nc) as tc:
        with (
            tc.tile_pool(name="const", bufs=1) as cp,
            tc.tile_pool(name="stage", bufs=2) as stp,
            tc.tile_pool(name="edgeg", bufs=2) as epg,
            tc.tile_pool(name="edges", bufs=2) as eps,
            tc.tile_pool(name="ps", bufs=1, space="PSUM") as pp,
            tc.tile_pool(name="dram", bufs=1, space="DRAM") as dp,
        ):
            dma_sems1 = [nc.alloc_semaphore(f"gdma1_{b}") for b in range(PA1)]
            dma_sems2 = [nc.alloc_semaphore(f"gdma2_{b}") for b in range(PA2)]
            for _s in dma_sems1 + dma_sems2:
                nc.gpsimd.sem_clear(_s)
            hbuf_writes = []
            h2_writes = []

            # ---- constants ----
            w1a_sb = cp.tile([P, HC + 16], dt.bfloat16)
            nc.sync.dma_start(w1a_sb[:], W1a[:, :])
            w2a_sb = cp.tile([P, 2, OUT_CH + 2], dt.bfloat16)
            nc.sync.dma_start(w2a_sb[:],
                              W2a[:, :].rearrange("(k p) n -> p k n", p=P))
            b1_sb = cp.tile([P, HC], dt.float32)
            nc.sync.dma_start(b1_sb[:], b1r[:, :])
            b2_sb = cp.tile([P, OUT_CH], dt.float32)
            nc.sync.dma_start(b2_sb[:], b2r[:, :])
            idx_sb = cp.tile([P, 8 * NCH], dt.int16)
            nc.sync.dma_start(idx_sb[:], idxT[:, :])
            idx2_sb = cp.tile([P, 8 * NCH], dt.int16)
            nc.sync.dma_start(idx2_sb[:], idx2T[:, :])

            iot_row = cp.tile([P, P], dt.float32)
            nc.gpsimd.iota(iot_row[:], pattern=[[1, P]], base=0,
                           channel_multiplier=0,
                           allow_small_or_imprecise_dtypes=True)
            iot_col = cp.tile([P, 1], dt.float32)
            nc.gpsimd.iota(iot_col[:], pattern=[[0, 1]], base=0,
                           channel_multiplier=1,
                           allow_small_or_imprecise_dtypes=True)
            ident = cp.tile([P, P], dt.bfloat16)
            nc.vector.tensor_scalar(ident[:], iot_row[:], iot_col[:], None,
                                    op0=OP.is_equal)

            Hbuf = dp.tile([N_PAD, ROW1], dt.bfloat16)
            h2r = [dp.tile([TS[k] * P, ROW2], dt.bfloat16, name=f"h2r{k}")
                   for k in range(NCHUNK)]
            h2g = [dp.tile([N_CORES * TS[k] * P, ROW2], dt.bfloat16,
                           name=f"h2g{k}", addr_space="Shared")
                   for k in range(NCHUNK)]
            h2all = dp.tile([N_PAD, ROW2], dt.bfloat16)

            # ---- gather prep helpers (desc-gen decoupled from firing) ----
            hg_tiles = {}
            hg2_tiles = {}

            def emit_prep1(j):
                Gj = G[j]
                ni = Gj * P
                hg = epg.tile([P, Gj, ROW1], dt.bfloat16, tag="hg", bufs=PA1,
                              padded_shape=[P, Gmax, ROW1], name=f"hg{j}")
                hg_tiles[j] = hg
                if not PREP_PIPELINE:
                    return
                nc.gpsimd.dma_gather(
                    hg[:, 0:Gj, :], Hbuf[:, :],
                    idx_sb[:, 8 * coff[j]:8 * (coff[j] + Gj)],
                    num_idxs=ni, num_idxs_reg=ni, elem_size=ROW1,
                    single_packet=SINGLE_PACKET, prepare_only=True,
                    sem=dma_sems1[j % PA1], queue_num=0)

            def emit_prep2(j):
                Gj = G[j]
                ni = Gj * P
                hg2 = epg.tile([P, Gj, ROW2], dt.bfloat16, tag="hg2", bufs=PA2,
                               padded_shape=[P, Gmax, ROW2], name=f"hg2_{j}")
                hg2_tiles[j] = hg2
                if not PREP2:
                    return
                nc.gpsimd.dma_gather(
                    hg2[:, 0:Gj, :], h2all[:, :],
                    idx2_sb[:, 8 * coff[j]:8 * (coff[j] + Gj)],
                    num_idxs=ni, num_idxs_reg=ni, elem_size=ROW2,
                    single_packet=SINGLE_PACKET, prepare_only=True,
                    sem=dma_sems2[j % PA2], queue_num=1)

            n_prep1 = 0
            n_prep2 = 0
            pend = [0, 0]

            # ---- phase A: GEMM1 for all node tiles -> Hbuf rows ----
            xloc_sb = cp.tile([P, LOC_NODES], dt.bfloat16)
            nc.sync.dma_start(xloc_sb[:], xTloc[:, :])

            GRP = 16
            hview = Hbuf[:, :].rearrange("(m p) r -> p m r", p=P)
            for grp in range((N_TILES_TOTAL + GRP - 1) // GRP):
                n_in = min(GRP, N_TILES_TOTAL - grp * GRP)
                xch = stp.tile([P, GRP * P], dt.bfloat16, tag="xch")
                nc.sync.dma_start(xch[:, 0:n_in * P],
                                  xT[:, grp * GRP * P:(grp * GRP + n_in) * P])
                stg = stp.tile([P, GRP, ROW1], dt.bfloat16, tag="stgA")
                for k in range(n_in):
                    t = grp * GRP + k
                    psA = pp.tile([P, HC + 16], dt.float32, tag="psA", bufs=3)
                    nc.tensor.matmul(psA[:],
                                     xch[:, k * P:(k + 1) * P],
                                     w1a_sb[:], start=True, stop=True)
                    if t % 2 == 0:
                        nc.scalar.copy(stg[:, k, 0:ROW1D], psA[:, 0:ROW1D])
                    else:
                        nc.vector.tensor_copy(stg[:, k, 0:ROW1D],
                                              psA[:, 0:ROW1D])
                hbuf_writes.append(nc.sync.dma_start(
                    hview[:, grp * GRP:grp * GRP + n_in, :],
                    stg[:, 0:n_in, :]).ins)

            # ---- local asrc/adst (per-core xTloc) ----
            loc_a = cp.tile([P, TILES_PER_CORE, 16], dt.bfloat16)
            for jh in range(TILES_PER_CORE):
                psL = pp.tile([P, HC + 16], dt.float32, tag="psA", bufs=3)
                nc.tensor.matmul(psL[:, 0:16],
                                 xloc_sb[:, jh * P:(jh + 1) * P],
                                 w1a_sb[:, HC:HC + 16], start=True, stop=True)
                nc.vector.tensor_copy(loc_a[:, jh:jh + 1, :],
                                      psL[:, 0:16].unsqueeze(1))

            loc_adst2 = cp.tile([P, TILES_PER_CORE, 1], dt.bfloat16)

            for _j in range(PA1):
                emit_prep1(_j)
            n_prep1 = PA1
            pend[0] = PA1

            # ---- phase B: layer-1 edges (+ chunked allgather of h2 rows) ----
            for j in range(TILES_PER_CORE):
                Gj = G[j]
                hg = hg_tiles[j]
                if PREP_PIPELINE:
                    if pend[0]:
                        nc.gpsimd.trigger_dma(count=None, queue_num=0)
                        pend[0] = 0
                else:
                    ni = Gj * P
                    nc.gpsimd.dma_gather(
                        hg[:, 0:Gj, :], Hbuf[:, :],
                        idx_sb[:, 8 * coff[j]:8 * (coff[j] + Gj)],
                        num_idxs=ni, num_idxs_reg=ni, elem_size=ROW1,
                        single_packet=SINGLE_PACKET)

                t01 = eps.tile([P, Gj, P], dt.bfloat16, tag="t01",
                               padded_shape=[P, Gmax, P])
                nc.sync.dma_start(t01[:, 0:Gj, :],
                                  t01T[:, coff[j] * P:(coff[j] + Gj) * P]
                                  .rearrange("p (g e) -> p g e", e=P))
                s01 = eps.tile([P, Gj, P], dt.bfloat16, tag="s01",
                               padded_shape=[P, Gmax, P])
                nc.sync.dma_start(s01[:, 0:Gj, :],
                                  s01T[:, coff[j] * P:(coff[j] + Gj) * P]
                                  .rearrange("p (g e) -> p g e", e=P))

                ps_ad = pp.tile([P, Gj, 8], dt.float32, tag="ps_ad", bufs=2,
                                padded_shape=[P, Gmax, 8])
                for g in range(Gj):
                    nc.tensor.matmul(ps_ad[:, g, :], t01[:, g, :],
                                     loc_a[:, j, 8:16], start=True, stop=True)
                if PREP_PIPELINE:
                    nc.vector.wait_ge(dma_sems1[j % PA1],
                                      16 * (j // PA1 + 1))
                er = stp.tile([P, Gj, 8], dt.float32, tag="er",
                              padded_shape=[P, Gmax, 8])
                nc.vector.tensor_tensor(er[:], hg[:, 0:Gj, 256:264],
                                        ps_ad[:], op=OP.add)
                er2 = stp.tile([P, Gj, 8], dt.float32, tag="er2",
                               padded_shape=[P, Gmax, 8])
                nc.vector.scalar_tensor_tensor(er2[:], er[:], NEG_SLOPE, er[:],
                                               op0=OP.mult, op1=OP.max)
                alp = stp.tile([P, Gj, 8], dt.float32, tag="alp",
                               padded_shape=[P, Gmax, 8])
                nc.scalar.activation(alp[:], er2[:], AF.Exp)
                alpb = stp.tile([P, Gj, 8], dt.bfloat16, tag="alpb",
                                padded_shape=[P, Gmax, 8])
                nc.scalar.copy(alpb[:], alp[:])
                mg = eps.tile([P, Gj, ROW1D], dt.bfloat16, tag="mg",
                              padded_shape=[P, Gmax, ROW1D])
                nc.vector.tensor_tensor(
                    mg[:, :, 0:HC].rearrange("p g (h c) -> p g h c", h=HEADS),
                    hg[:, 0:Gj, 0:HC].rearrange("p g (h c) -> p g h c", h=HEADS),
                    alpb[:].unsqueeze(3).broadcast_to([P, Gj, 8, HID]),
                    op=OP.mult)
                nc.scalar.copy(mg[:, :, HC:HC + 8], alpb[:])

                ps_o = pp.tile([P, HC + 8], dt.float32, tag="ps_o", bufs=2)
                for g in range(Gj):
                    nc.tensor.matmul(ps_o[:], s01[:, g, :], mg[:, g, :],
                                     start=(g == 0), stop=(g == Gj - 1))

                if n_prep1 < TILES_PER_CORE:
                    emit_prep1(n_prep1)
                    n_prep1 += 1
                    pend[0] += 1


                rec = stp.tile([P, 8], dt.float32, tag="rec")
                nc.vector.reciprocal(rec[:], ps_o[:, HC:HC + 8])
                o1 = stp.tile([P, HC], dt.float32, tag="o1")
                nc.vector.tensor_tensor(
                    o1[:].rearrange("p (h c) -> p h c", h=HEADS),
                    ps_o[:, 0:HC].rearrange("p (h c) -> p h c", h=HEADS),
                    rec[:].unsqueeze(2).broadcast_to([P, 8, HID]),
                    op=OP.mult)
                o1b = stp.tile([P, HC], dt.float32, tag="o1b")
                nc.vector.tensor_tensor(o1b[:], o1[:], b1_sb[:], op=OP.add)
                # ELU(x) = max(x,0) + exp(min(x,0)) - 1
                en = stp.tile([P, HC], dt.float32, tag="en")
                nc.vector.tensor_scalar(en[:], o1b[:], 0.0, None, op0=OP.min)
                ex = stp.tile([P, HC], dt.float32, tag="ex")
                nc.scalar.activation(ex[:], en[:], AF.Exp)
                h2a = stp.tile([P, HC], dt.float32, tag="h2a")
                nc.vector.scalar_tensor_tensor(h2a[:], o1b[:], 0.0, ex[:],
                                               op0=OP.max, op1=OP.add)
                h2 = stp.tile([P, HC], dt.bfloat16, tag="h2")
                nc.vector.tensor_scalar(h2[:], h2a[:], 1.0, None,
                                        op0=OP.subtract)

                h2T = stp.tile([P, 2, P], dt.bfloat16, tag="h2T")
                for k in range(2):
                    pst = pp.tile([P, HC + 16], dt.float32, tag="psA", bufs=3)
                    pstb = pst[:].bitcast(dt.bfloat16)
                    nc.tensor.transpose(pstb[:, 0:P], h2[:, k * P:(k + 1) * P],
                                        ident[:])
                    nc.scalar.copy(h2T[:, k, :], pstb[:, 0:P])
                psw = pp.tile([P, HC + 16], dt.float32, tag="psA", bufs=3)
                ps2 = psw[:, 0:OUT_CH + 2]
                for k in range(2):
                    nc.tensor.matmul(ps2, h2T[:, k, :], w2a_sb[:, k, :],
                                     start=(k == 0), stop=(k == 1))
                row2 = stp.tile([P, ROW2D], dt.bfloat16, tag="row2")
                nc.scalar.copy(row2[:, 0:ROW2D], ps2[:, 0:ROW2D])
                nc.vector.tensor_copy(loc_adst2[:, j, :],
                                      ps2[:, OUT_CH + 1:OUT_CH + 2])
                kc = 0
                while CUM[kc + 1] <= j:
                    kc += 1
                nc.sync.dma_start(
                    h2r[kc][:, :].rearrange("(t p) r -> p t r", p=P)
                    [:, j - CUM[kc], 0:ROW2D],
                    row2[:, :])

                # Emit chunk-k's collective 1-2 tiles after its rows
                # complete, so its semaphore wait is pre-satisfied and does
                # not head-of-line-block the next tile's gather on the
                # gpsimd queue. The last chunk has no slack.
                for k in range(NCHUNK):
                    due = min(CUM[k + 1] + 1, TILES_PER_CORE - 1)
                    if j == due:
                        nc.gpsimd.collective_compute(
                            "AllGather", OP.bypass,
                            replica_groups=[list(range(N_CORES))],
                            ins=[h2r[k][:, :].opt()],
                            outs=[h2g[k][:, :].opt()])
                        h2_writes.append(nc.sync.dma_start(
                            h2all[CUM[k] * N_CORES * P:
                                  CUM[k + 1] * N_CORES * P, :],
                            h2g[k][:, :]).ins)

            if PREP2:
                while n_prep2 < 6:
                    emit_prep2(n_prep2)
                    n_prep2 += 1
                    pend[1] += 1

            # ---- phase D: layer-2 edges ----
            for j in range(TILES_PER_CORE):
                Gj = G[j]
                if PREP2:
                    hg2 = hg2_tiles[j]
                    if pend[1]:
                        nc.gpsimd.trigger_dma(count=None, queue_num=1)
                        pend[1] = 0
                else:
                    while n_prep2 < min(TILES_PER_CORE, j + PA2):
                        emit_prep2(n_prep2)
                        n_prep2 += 1
                    hg2 = hg2_tiles[j]
                    ni = Gj * P
                    nc.gpsimd.dma_gather(
                        hg2[:, 0:Gj, :], h2all[:, :],
                        idx2_sb[:, 8 * coff[j]:8 * (coff[j] + Gj)],
                        num_idxs=ni, num_idxs_reg=ni, elem_size=ROW2,
                        single_packet=SINGLE_PACKET)

                t01 = eps.tile([P, Gj, P], dt.bfloat16, tag="t01",
                               padded_shape=[P, Gmax, P])
                nc.sync.dma_start(t01[:, 0:Gj, :],
                                  t01T[:, coff[j] * P:(coff[j] + Gj) * P]
                                  .rearrange("p (g e) -> p g e", e=P))
                s01 = eps.tile([P, Gj, P], dt.bfloat16, tag="s01",
                               padded_shape=[P, Gmax, P])
                nc.sync.dma_start(s01[:, 0:Gj, :],
                                  s01T[:, coff[j] * P:(coff[j] + Gj) * P]
                                  .rearrange("p (g e) -> p g e", e=P))

                ps_a2f = pp.tile([P, Gj, 8], dt.float32, tag="ps_ad", bufs=2,
                                 padded_shape=[P, Gmax, 8])
                ps_a2 = ps_a2f[:, :, 0:1]
                for g in range(Gj):
                    nc.tensor.matmul(ps_a2[:, g, :], t01[:, g, :],
                                     loc_adst2[:, j, :], start=True, stop=True)
                if PREP2:
                    nc.vector.wait_ge(dma_sems2[j % PA2],
                                      16 * (j // PA2 + 1))
                e2 = stp.tile([P, Gj, 1], dt.float32, tag="e2",
                              padded_shape=[P, Gmax, 1])
                nc.vector.tensor_tensor(e2[:],
                                        hg2[:, 0:Gj, OUT_CH:OUT_CH + 1],
                                        ps_a2[:], op=OP.add)
                e2b = stp.tile([P, Gj, 1], dt.float32, tag="e2b",
                               padded_shape=[P, Gmax, 1])
                nc.vector.scalar_tensor_tensor(e2b[:], e2[:], NEG_SLOPE, e2[:],
                                               op0=OP.mult, op1=OP.max)
                al2 = stp.tile([P, Gj, 1], dt.float32, tag="al2",
                               padded_shape=[P, Gmax, 1])
                nc.scalar.activation(al2[:], e2b[:], AF.Exp)
                al2b = stp.tile([P, Gj, 1], dt.bfloat16, tag="al2b",
                                padded_shape=[P, Gmax, 1])
                nc.scalar.copy(al2b[:], al2[:])
                mg2 = eps.tile([P, Gj, OUT_CH + 1], dt.bfloat16, tag="mg2",
                               padded_shape=[P, Gmax, OUT_CH + 1])
                nc.vector.tensor_tensor(mg2[:, :, 0:OUT_CH],
                                        hg2[:, 0:Gj, 0:OUT_CH],
                                        al2b[:].broadcast_to([P, Gj, OUT_CH]),
                                        op=OP.mult)
                nc.scalar.copy(mg2[:, :, OUT_CH:OUT_CH + 1], al2b[:])

                ps3f = pp.tile([P, HC + 8], dt.float32, tag="ps_o", bufs=2)
                ps3 = ps3f[:, 0:OUT_CH + 1]
                for g in range(Gj):
                    nc.tensor.matmul(ps3, s01[:, g, :], mg2[:, g, :],
                                     start=(g == 0), stop=(g == Gj - 1))
                rec2 = stp.tile([P, 1], dt.float32, tag="rec2")
                nc.vector.reciprocal(rec2[:], ps3[:, OUT_CH:OUT_CH + 1])
                o2 = stp.tile([P, OUT_CH], dt.float32, tag="o2")
                nc.vector.tensor_tensor(o2[:], ps3[:, 0:OUT_CH],
                                        rec2[:].broadcast_to([P, OUT_CH]),
                                        op=OP.mult)
                o2b = stp.tile([P, OUT_CH], dt.float32, tag="o2b")
                nc.vector.tensor_tensor(o2b[:], o2[:], b2_sb[:], op=OP.add)
                nc.sync.dma_start(
                    outT[:, :].rearrange("(t p) r -> p t r", p=P)[:, j, :],
                    o2b[:])
                if PREP2 and n_prep2 < TILES_PER_CORE:
                    emit_prep2(n_prep2)
                    n_prep2 += 1
                    pend[1] += 1

    nc.compile()
    return nc


def kernel(x, edge_index, W1, a_src1, a_dst1, b1, W2, a_src2, a_dst2, b2):
    x = np.asarray(x, dtype=np.float32)
    W1 = np.asarray(W1, dtype=np.float32)
    a_src1 = np.asarray(a_src1, dtype=np.float32)
    a_dst1 = np.asarray(a_dst1, dtype=np.float32)
    b1 = np.asarray(b1, dtype=np.float32)
    W2 = np.asarray(W2, dtype=np.float32)
    a_src2 = np.asarray(a_src2, dtype=np.float32)
    a_dst2 = np.asarray(a_dst2, dtype=np.float32)
    b2 = np.asarray(b2, dtype=np.float32)

    G, meta = _prep_edges(edge_index)

    A1 = np.zeros((HC, 16), np.float32)
    for h in range(HEADS):
        A1[h * HID:(h + 1) * HID, h] = a_src1[h]
        A1[h * HID:(h + 1) * HID, 8 + h] = a_dst1[h]
    W1a = np.ascontiguousarray(
        np.concatenate([W1, W1 @ A1], axis=1)).astype(bf16)
    W2a = np.ascontiguousarray(
        np.concatenate([W2, W2 @ a_src2.T, W2 @ a_dst2.T], axis=1)).astype(bf16)

    xT = np.zeros((P, N_PAD), bf16)
    xT[:, :N_NODES] = x.T.astype(bf16)
    b1r = np.ascontiguousarray(np.tile(b1[None, :], (P, 1)).astype(np.float32))
    b2r = np.ascontiguousarray(np.tile(b2[None, :], (P, 1)).astype(np.float32))

    nc = _build_program(G)

    in_maps = []
    for c in range(N_CORES):
        in_maps.append({
            "xT": xT,
            "xTloc": np.ascontiguousarray(
                xT[:, c * LOC_NODES:(c + 1) * LOC_NODES]),
            "W1a": W1a, "W2a": W2a, "b1r": b1r, "b2r": b2r,
            "idx": meta[c]["idx"], "idx2": meta[c]["idx2"],
            "s01": meta[c]["s01"], "t01": meta[c]["t01"],
        })

    import os
    kw = {}
    if os.environ.get("GAT_TRACE"):
        kw = dict(trace=True, trace_cores=[0])
    res = run_bass_kernel_spmd(nc, in_maps, core_ids=list(range(N_CORES)), **kw)
    global LAST_RESULTS
    LAST_RESULTS = res
    out = np.concatenate([res.results[c]["out"] for c in range(N_CORES)], axis=0)
    return out[:N_NODES]


LAST_RESULTS = None
